# revision 1
# baseline (speedup 1.0000x reference)
"""Trainium2 Bass kernel for MultiHeadAttention with relative-position bias.

Problem shapes: N=4, S=1024, H=1024, NH=16, D=64, P=20 (clamp window).
Returns (out, ctx) like the reference.

Sharding: 8 cores; core c handles batch n=c//2, head-group hg=c%2 (8 heads).
Each core computes its heads' QKV projections, attention, the ctx column
slice, and a partial out (row-sharded Wo contraction). Host sums the two
partials per batch and adds bo.

Device-side structure:
  - Inputs arrive host-transposed (h-major) so projections contract over h
    directly; matmuls run in float32r (1 cycle/row at moving>=256); the
    attention-weight/V path runs in bf16 (random rounding averages out).
  - energy_pos[q,k] = Q[q]*rel_emb[clip(q-k,-20,20)+20]: B = Q @ rel_emb^T
    rides in the same PSUM tile as QK^T; the far-field column enters the
    fused exp as a per-partition bias; the 20-wide near-diagonal correction
    is placed by a diagonal-access-pattern DMA onto a causal-mask template.
  - Softmax without max-subtraction (energies are O(0.3)); the row sum is
    fused into the exp (accum_out); normalization is deferred to the
    per-partition-scaled ctx eviction in phase C.
  - P is transposed on the PE in q-block pairs for wide AV matmuls; ctx^T
    is re-transposed once more after normalization to feed the output
    projection with a 128-deep contraction.
"""

import sys

if "/opt/trn_rl_repo" not in sys.path:
    sys.path.insert(0, "/opt/trn_rl_repo")

import numpy as np

import concourse.bass as bass
import concourse.mybir as mybir
import concourse.tile as tile
from concourse import bacc
from concourse.bass_utils import run_bass_kernel_spmd

F32 = mybir.dt.float32
F32R = mybir.dt.float32r
AF = mybir.ActivationFunctionType

S = 1024
D = 64
NHG = 8      # heads per core
NPAIR = 4    # head pairs per core
HC = 8       # 128-row contraction chunks over H
SB = 8       # 128-row blocks over S
BCOL = 982   # column where the 42 B-columns live inside the S psum tile
MASKV = -1.0e9
WIN = 147    # band window width (19 + 128)


def _chunks(w):
    out = [(0, min(w, 512))]
    if w > 512:
        out.append((512, w))
    return out


def build_nc():
    nc = bacc.Bacc("TRN2", target_bir_lowering=False, debug=False)

    xqT = nc.dram_tensor("xqT", (S, S), F32R, kind="ExternalInput").ap()
    xkT = nc.dram_tensor("xkT", (S, S), F32R, kind="ExternalInput").ap()
    xvT = nc.dram_tensor("xvT", (S, S), F32R, kind="ExternalInput").ap()
    wq = nc.dram_tensor("wq", (S, 512), F32R, kind="ExternalInput").ap()
    wk = nc.dram_tensor("wk", (S, 512), F32R, kind="ExternalInput").ap()
    wv = nc.dram_tensor("wv", (S, 512), F32R, kind="ExternalInput").ap()
    wo = nc.dram_tensor("wo", (512, S), F32R, kind="ExternalInput").ap()
    bq2 = nc.dram_tensor("bq2", (128, 4), F32, kind="ExternalInput").ap()
    bk2 = nc.dram_tensor("bk2", (128, 4), F32, kind="ExternalInput").ap()
    bvr = nc.dram_tensor("bvr", (1, 512), F32R, kind="ExternalInput").ap()
    relTr = nc.dram_tensor("relTr", (128, 42), F32R, kind="ExternalInput").ap()

    o_part = nc.dram_tensor("o_part", (S, S), F32, kind="ExternalOutput").ap()
    ctx_out = nc.dram_tensor("ctx_out", (S, 512), F32, kind="ExternalOutput").ap()

    import ml_dtypes
    ident_np = np.eye(128, dtype=np.float32)
    templ_np = np.zeros((128, WIN), dtype=np.float32)
    for p in range(128):
        templ_np[p, p + 20:] = MASKV
    templ_np = templ_np.astype(ml_dtypes.bfloat16)
    ident_d = nc.inline_tensor(ident_np, name="ident_c")
    identb_d = nc.inline_tensor(ident_np.astype(ml_dtypes.bfloat16),
                                name="identb_c")
    templ_d = nc.inline_tensor(templ_np, name="templ_c")
    ones_d = nc.inline_tensor(np.ones((1, 128), np.float32), name="ones_c")
    zeros_d = nc.inline_tensor(np.zeros((128, 128), np.float32),
                               name="zeros_c")

    BF16 = mybir.dt.bfloat16

    # greedy ACT/DVE balance for PSUM->SBUF evictions.
    # Pre-loaded with the fixed per-engine work (ACT: exp ~56us;
    # DVE: band adds/src/Ball/recip ~30us) so copies land fairly.
    ebusy = {"act": 72000.0, "dve": 30000.0}

    def _pick(cact, cdve):
        if ebusy["act"] + cact < ebusy["dve"] + cdve:
            ebusy["act"] += cact
            return "act"
        ebusy["dve"] += cdve
        return "dve"

    def ecopy(out, in_, cols):
        if _pick(cols * 0.833 + 280.0, cols * 1.042 + 170.0) == "act":
            nc.scalar.copy(out, in_)
        else:
            nc.vector.tensor_copy(out, in_)

    def escale(out, in_, scale, cols):
        if _pick(cols * 0.833 + 280.0, cols * 1.042 + 170.0) == "act":
            nc.scalar.activation(out, in_, AF.Copy, scale=scale)
        else:
            nc.vector.tensor_scalar_mul(out, in_, scale)

    def ebias(out, in_, bias, cols):
        if _pick(cols * 0.833 + 280.0, cols * 1.042 + 170.0) == "act":
            nc.scalar.activation(out, in_, AF.Identity, bias=bias)
        else:
            nc.vector.tensor_scalar_add(out, in_, bias)

    with tile.TileContext(nc) as tc:
        import contextlib

        with contextlib.ExitStack() as ctx:
            ep = ctx.enter_context
            cpool = ep(tc.tile_pool(name="consts", bufs=1))
            ident = cpool.tile([128, 128], F32R, tag="ident")
            nc.sync.dma_start(ident[:], ident_d.ap().bitcast(F32R))
            templ = cpool.tile([128, WIN], BF16, tag="templ")
            nc.sync.dma_start(templ[:], templ_d.ap())
            relT = cpool.tile([128, 42], F32R, tag="relT")
            nc.sync.dma_start(relT[:], relTr)
            bq_sb = cpool.tile([128, 4], F32, tag="bq")
            nc.sync.dma_start(bq_sb[:], bq2)
            bk_sb = cpool.tile([128, 4], F32, tag="bk")
            nc.sync.dma_start(bk_sb[:], bk2)
            bv_sb = cpool.tile([1, 512], F32R, tag="bv")
            nc.sync.dma_start(bv_sb[:], bvr)
            ones = cpool.tile([1, 128], F32R, tag="ones")
            nc.sync.dma_start(ones[:], ones_d.ap().bitcast(F32R))
            zero128 = cpool.tile([128, 128], BF16, tag="zero128")
            nc.sync.dma_start(zero128[:],
                              zeros_d.ap().bitcast(BF16)[:, 0:128])
            identb = cpool.tile([128, 128], BF16, tag="identb")
            nc.sync.dma_start(identb[:], identb_d.ap())

            big = ep(tc.tile_pool(name="big", bufs=1))
            qT = big.tile([128, NPAIR, S], F32R, tag="qT", name="qT")[:]
            kT = big.tile([128, NPAIR, S], F32R, tag="kT", name="kT")[:]
            vN = big.tile([128, SB, 512], BF16, tag="vN", name="vN")[:]
            stg_h = []
            for _i in range(NHG):
                _t = big.tile([128, SB, WIN], BF16, tag=f"stg{_i}",
                              name=f"stg{_i}")
                stg_h.append(_t[:])
            bias2 = big.tile([128, NHG * SB], F32, tag="bias2",
                             name="bias2")[:]

            # PSUM pools: 2*2 (S) + 3 (general) + 1 (AV) = 8 banks
            spp = ep(tc.tile_pool(name="spp", bufs=2, space="PSUM"))
            gpp = ep(tc.tile_pool(name="gpp", bufs=3, space="PSUM"))
            cxp = ep(tc.tile_pool(name="cxp", bufs=1, space="PSUM"))

            # SBUF working pools (coexist with xT/wx below)
            pbuf = ep(tc.tile_pool(name="pbuf", bufs=8))
            ptbuf = ep(tc.tile_pool(name="ptbuf", bufs=3))
            cujp = ep(tc.tile_pool(name="cujp", bufs=2))
            cns = ep(tc.tile_pool(name="cns", bufs=2))
            ctp = ep(tc.tile_pool(name="ctp", bufs=2))
            osb = ep(tc.tile_pool(name="osb", bufs=2))
            small = ep(tc.tile_pool(name="small", bufs=4))
            bsm = ep(tc.tile_pool(name="bsm", bufs=12))
            xTp = ep(tc.tile_pool(name="xTp", bufs=1))
            wxp = ep(tc.tile_pool(name="wxp", bufs=2))

            # ---------------- Phase A: loads + projections + pre-pass -------
            def load_input(xdram, wdram):
                w_sb = wxp.tile([128, HC, 512], F32R, tag="wx", name="w_sb")[:]
                nc.sync.dma_start(
                    w_sb, wdram.rearrange("(c p) n -> p c n", p=128))
                xT = xTp.tile([128, HC, S], F32R, tag="xT", name="xT")[:]
                for hc in range(HC):
                    nc.sync.dma_start(xT[:, hc, :],
                                      xdram[hc * 128:(hc + 1) * 128, :])
                return xT, w_sb

            def proj_qk(xT, w_sb, outT, b_sb):
                for pair in range(NPAIR):
                    for qc in range(2):
                        pp = gpp.tile([128, 512], F32, tag="gp", name="pp")
                        for hc in range(HC):
                            nc.tensor.matmul(
                                pp[:],
                                w_sb[:, hc, pair * 128:(pair + 1) * 128],
                                xT[:, hc, qc * 512:(qc + 1) * 512],
                                start=(hc == 0), stop=(hc == HC - 1))
                        ebias(outT[:, pair, qc * 512:(qc + 1) * 512],
                              pp[:], b_sb[:, pair:pair + 1], 512)

            # Q first (pre-pass depends on it); K rides in the P-pool
            # slots (same shape, idle until attention) so its load is not
            # serialized behind the xT slot.
            xTq, w_q = load_input(xqT, wq)
            xkc = []
            for hc in range(HC):
                xk1 = pbuf.tile([128, 1024], F32R, tag="P", name=f"xk{hc}")
                nc.sync.dma_start(xk1[:], xkT[hc * 128:(hc + 1) * 128, :])
                xkc.append(xk1[:])
            w_k = wxp.tile([128, HC, 512], F32R, tag="wx", name="w_k")[:]
            nc.sync.dma_start(w_k, wk.rearrange("(c p) n -> p c n", p=128))

            proj_qk(xTq, w_q, qT, bq_sb)

            # fill all staging tiles with the causal-mask template up front
            for h in range(NHG):
                for t in range(SB):
                    nc.gpsimd.tensor_copy(stg_h[h][:, t, :], templ[:])

            # band pre-pass: B = Q @ rel^T, staging tiles + biases
            def prepass(ts_):
                for t in ts_:
                    for h in range(NHG):
                        pairb, halfb = divmod(h, 2)
                        idx = h * SB + t
                        bp = gpp.tile([128, 512], F32, tag="gp", name="bp")
                        nc.tensor.matmul(
                            bp[:, 0:42],
                            qT[64 * halfb:64 * halfb + 64, pairb,
                               t * 128:(t + 1) * 128],
                            relT[64 * halfb:64 * halfb + 64, :],
                            start=True, stop=True)
                        nc.vector.tensor_scalar_mul(
                            bias2[:, idx:idx + 1], bp[:, 0:1], 0.125)
                        srcb = bsm.tile([128, 20], BF16, tag="srcb")
                        nc.vector.tensor_scalar(
                            srcb[:], bp[:, 1:21], bp[:, 0:1], 8.0,
                            mybir.AluOpType.subtract,
                            mybir.AluOpType.mult)
                        stga = stg_h[h][:, t, :]
                        diag = bass.AP(
                            stga.tensor, stga.offset,
                            [[SB * WIN + 1, 128], [1, 20]])
                        if idx % 2 == 0:
                            nc.sync.dma_start(diag, srcb[:])
                        else:
                            nc.gpsimd.dma_start(diag, srcb[:])

            # K projection from the P-slot chunks
            for pair in range(NPAIR):
                for qc in range(2):
                    pp = gpp.tile([128, 512], F32, tag="gp", name="pp")
                    for hc in range(HC):
                        nc.tensor.matmul(
                            pp[:],
                            w_k[:, hc, pair * 128:(pair + 1) * 128],
                            xkc[hc][:, qc * 512:(qc + 1) * 512],
                            start=(hc == 0), stop=(hc == HC - 1))
                    ebias(kT[:, pair, qc * 512:(qc + 1) * 512],
                          pp[:], bk_sb[:, pair:pair + 1], 512)

            prepass((0, 1))

            # V projection
            xTv, w_v = load_input(xvT, wv)
            for kb in range(SB):
                pp = gpp.tile([128, 512], F32, tag="gp", name="pp")
                for hc in range(HC):
                    nc.tensor.matmul(
                        pp[:],
                        xTv[:, hc, kb * 128:(kb + 1) * 128],
                        w_v[:, hc, :],
                        start=(hc == 0), stop=False)
                nc.tensor.matmul(pp[:], ones[:], bv_sb[:],
                                 start=False, stop=True)
                ecopy(vN[:, kb, :], pp[:], 512)

            prepass((2, 3))

            # wo reuses the (now free) xT slot
            wo_sb = xTp.tile([128, NPAIR, S], F32R, tag="xT",
                             name="wo_sb")[:]
            nc.sync.dma_start(wo_sb, wo.rearrange("(c p) n -> p c n", p=128))

            # ------ attention per q-pair j, staged batches with lookahead ---
            state = {}

            def s_batch(j, hb):
                rj, cuj, Pt = state[j]
                for h in (hb, hb + 1):
                    pair, half = divmod(h, 2)
                    qTh = qT[64 * half:64 * half + 64]
                    kTh = kT[64 * half:64 * half + 64]
                    for tt in (0, 1):
                        t = 2 * j + tt
                        W = 128 * (t + 1)
                        idx = h * SB + t
                        sp = spp.tile([128, 1024], F32, tag="sp", name="sp")
                        lhs = qTh[:, pair, t * 128:(t + 1) * 128]
                        for c0, c1 in _chunks(W):
                            nc.tensor.matmul(sp[:, c0:c1], lhs,
                                             kTh[:, pair, c0:c1],
                                             start=True, stop=True)
                        stga = stg_h[h][:, t, :]
                        if t == 0:
                            nc.vector.tensor_add(
                                sp[:, 0:128], sp[:, 0:128], stga[:, 19:WIN])
                        else:
                            w0 = t * 128 - 19
                            nc.vector.tensor_add(
                                sp[:, w0:w0 + WIN], sp[:, w0:w0 + WIN],
                                stga[:, :])
                        P = pbuf.tile([128, 1024], BF16, tag="P", name="P")
                        sums = small.tile([128, 1], F32, tag="sums")
                        nc.scalar.activation(
                            P[:, 0:W], sp[:, 0:W], AF.Exp,
                            bias=bias2[:, idx:idx + 1],
                            scale=1.0 / 64.0, accum_out=sums[:])
                        nc.vector.reciprocal(rj[:, h, tt:tt + 1], sums[:])
                        Pt[(h, tt)] = P

            def t_batch(j, hb):
                rj, cuj, Pt = state[j]
                for h in (hb, hb + 1):
                    P0 = Pt.pop((h, 0))
                    P1 = Pt.pop((h, 1))
                    pT = ptbuf.tile([128, 2 * j + 2, 256], BF16,
                                    tag="pT", name="pT")[:]
                    Pt[("pT", h)] = pT
                    for kb2 in range(0, 2 * j + 2, 2):
                        pt = gpp.tile([128, 512], BF16, tag="gp", name="pt")
                        for i in range(2):
                            kb = kb2 + i
                            # kb == 2j+1: zeros into the unread quarter so
                            # the evict stays one 512-col copy
                            src0 = (P0[:, kb * 128:(kb + 1) * 128]
                                    if kb <= 2 * j else zero128[:])
                            nc.tensor.transpose(
                                pt[:, i * 256:i * 256 + 128],
                                src0, identb[:])
                            nc.tensor.transpose(
                                pt[:, i * 256 + 128:i * 256 + 256],
                                P1[:, kb * 128:(kb + 1) * 128],
                                identb[:])
                        ecopy(pT[:, kb2:kb2 + 2, :], pt[:, 0:512], 512)

            def av_batch(j, hb):
                rj, cuj, Pt = state[j]
                for h in (hb, hb + 1):
                    pT = Pt.pop(("pT", h))
                    cx = cxp.tile([64, 256], F32, tag="cx")
                    for kb in range(2 * j + 1):
                        nc.tensor.matmul(
                            cx[:], vN[:, kb, h * 64:(h + 1) * 64],
                            pT[:, kb, :],
                            start=(kb == 0), stop=False)
                    nc.tensor.matmul(
                        cx[:, 128:256],
                        vN[:, 2 * j + 1, h * 64:(h + 1) * 64],
                        pT[:, 2 * j + 1, 128:256],
                        start=False, stop=True)
                    ecopy(cuj[:, h, 0:256], cx[:], 256)

            def output_stage(j):
                rj, cuj, Pt = state.pop(j)
                for tt in (0, 1):
                    qb = 2 * j + tt
                    cnall = gpp.tile([128, 512], F32R, tag="gp", name="cnall")
                    for h in range(NHG):
                        nc.tensor.transpose(
                            cnall[:, h * 64:(h + 1) * 64],
                            cuj[:, h, tt * 128:(tt + 1) * 128],
                            ident[0:64, 0:64])
                    cn = cns.tile([128, 512], F32R, tag="cn")
                    for h in range(NHG):
                        escale(cn[:, h * 64:(h + 1) * 64],
                               cnall[:, h * 64:(h + 1) * 64],
                               rj[:, h, tt:tt + 1], 64)
                    nc.sync.dma_start(
                        ctx_out[qb * 128:(qb + 1) * 128, :].bitcast(F32R),
                        cn[:])
                    rt = gpp.tile([128, 512], F32R, tag="gp", name="rt")
                    for pc in range(NPAIR):
                        nc.tensor.transpose(
                            rt[:, pc * 128:(pc + 1) * 128],
                            cn[:, pc * 128:(pc + 1) * 128],
                            ident[:])
                    ctxT = ctp.tile([128, NPAIR, 128], F32R, tag="ctxT")
                    ecopy(ctxT[:], rt[:], 512)
                    ou = osb.tile([128, 1024], F32, tag="ou")
                    for oc in range(2):
                        op = spp.tile([128, 1024], F32, tag="sp", name="op")
                        for pc in range(NPAIR):
                            nc.tensor.matmul(
                                op[:, 0:512],
                                ctxT[:, pc, :],
                                wo_sb[:, pc, oc * 512:(oc + 1) * 512],
                                start=(pc == 0), stop=(pc == NPAIR - 1))
                        ecopy(ou[:, oc * 512:(oc + 1) * 512],
                              op[:, 0:512], 512)
                    nc.sync.dma_start(o_part[qb * 128:(qb + 1) * 128, :],
                                      ou[:])

            for j in range(NPAIR):
                if j == 1:
                    prepass((4, 5))
                elif j == 2:
                    prepass((6, 7))
                rj = cujp.tile([128, NHG, 2], F32, tag="rj", name="rj")[:]
                cuj = cujp.tile([64, NHG, 256], F32R, tag="cuj",
                                name="cuj")[:]
                state[j] = (rj, cuj, {})
                s_batch(j, 0)
                s_batch(j, 2)
                t_batch(j, 0)
                s_batch(j, 4)
                t_batch(j, 2)
                av_batch(j, 0)
                s_batch(j, 6)
                t_batch(j, 4)
                av_batch(j, 2)
                t_batch(j, 6)
                av_batch(j, 4)
                av_batch(j, 6)
                if j > 0:
                    output_stage(j - 1)
            output_stage(NPAIR - 1)

    nc.compile()
    return nc


_NC = None


def _get_nc():
    global _NC
    if _NC is None:
        _NC = build_nc()
    return _NC


def make_in_maps(query, key, value, Wq, bq, Wk, bk, Wv, bv, Wo, rel_emb):
    asf = lambda a: np.ascontiguousarray(a, dtype=np.float32)
    r1 = asf(rel_emb.T[:, ::-1])
    r1 = np.concatenate([r1, np.zeros((64, 1), np.float32)], axis=1)
    relTr = np.ascontiguousarray(np.concatenate([r1, r1], axis=0))
    in_maps = []
    for c in range(8):
        n, hg = divmod(c, 2)
        cs = slice(512 * hg, 512 * (hg + 1))
        in_maps.append({
            "xqT": asf(np.asarray(query[n]).T),
            "xkT": asf(np.asarray(key[n]).T),
            "xvT": asf(np.asarray(value[n]).T),
            "wq": asf(Wq[:, cs]),
            "wk": asf(Wk[:, cs]),
            "wv": asf(Wv[:, cs]),
            "wo": asf(Wo[cs, :]),
            "bq2": asf(np.asarray(bq)[cs].reshape(4, 128).T),
            "bk2": asf(np.asarray(bk)[cs].reshape(4, 128).T),
            "bvr": asf(np.asarray(bv)[cs].reshape(1, 512)),
            "relTr": relTr,
        })
    return in_maps


def run(inputs, trace=False, trace_kwargs=None):
    nc = _get_nc()
    in_maps = make_in_maps(
        np.asarray(inputs["query"]), np.asarray(inputs["key"]),
        np.asarray(inputs["value"]), np.asarray(inputs["Wq"]),
        np.asarray(inputs["bq"]), np.asarray(inputs["Wk"]),
        np.asarray(inputs["bk"]), np.asarray(inputs["Wv"]),
        np.asarray(inputs["bv"]), np.asarray(inputs["Wo"]),
        np.asarray(inputs["rel_emb"]))
    kw = {}
    if trace:
        kw["trace"] = True
        if trace_kwargs:
            kw.update(trace_kwargs)
    res = run_bass_kernel_spmd(nc, in_maps, core_ids=list(range(8)), **kw)
    bo = np.asarray(inputs["bo"], dtype=np.float32)
    out = np.zeros((4, S, S), np.float32)
    ctx = np.zeros((4, S, S), np.float32)
    for c in range(8):
        n, hg = divmod(c, 2)
        out[n] += res.results[c]["o_part"]
        ctx[n][:, 512 * hg:512 * (hg + 1)] = res.results[c]["ctx_out"]
    out += bo
    return (out, ctx), res


def kernel(**inputs):
    (out, ctx), _ = run(inputs)
    return (out, ctx)



# revision 58
# speedup vs baseline: 1.6367x; 1.6367x over previous
"""Trainium2 Bass kernel for MultiHeadAttention with relative-position bias.

Problem shapes: N=4, S=1024, H=1024, NH=16, D=64, P=20 (clamp window).
Returns (out, ctx) like the reference.

Sharding: 8 cores; core c handles batch n=c//2, head-group hg=c%2 (8 heads).
Each core computes its heads' QKV projections, attention, the ctx column
slice, and a partial out (row-sharded Wo contraction). Host sums the two
partials per batch and adds bo.

V3 design vs the fp32r baseline:
  - bf16 end to end: inputs, weights and outputs are uploaded/stored bf16
    (fp32 accumulation in PSUM), halving HBM traffic and enabling full-rate
    matmuls at any moving size.
  - P^T for the AV contraction comes from one XBAR dma_start_transpose per
    (head, q-tile) instead of 320 PE transposes + 80 PSUM evictions.
  - The far-field rel-position term is dropped entirely (it is constant
    across keys for a fixed query, so softmax cancels it); the 20-wide
    near-diagonal correction is placed by one diagonal-AP DMA per head onto
    causal-mask templates.
  - Softmax row sums via DVE tensor_reduce of P; normalization is a single
    broadcast multiply per q-tile at the ctx transpose.
  - PSUM->SBUF evictions are greedily balanced across ACT/DVE/GPSIMD.
"""

import sys

if "/opt/trn_rl_repo" not in sys.path:
    sys.path.insert(0, "/opt/trn_rl_repo")

import numpy as np

import concourse.bass as bass
import concourse.mybir as mybir
import concourse.tile as tile
from concourse import bacc
from concourse.bass_utils import run_bass_kernel_spmd

F32 = mybir.dt.float32
BF16 = mybir.dt.bfloat16
AF = mybir.ActivationFunctionType
ALU = mybir.AluOpType
AXIS = mybir.AxisListType

S = 1024
D = 64
NHG = 8      # heads per core
NPAIR = 4    # head pairs per core
HC = 8       # 128-row contraction chunks over H
SB = 8       # 128-row blocks over S
MASKV = -1.0e9
WIN = 147    # band window width (19 + 128)


def _chunks(w):
    out = [(0, min(w, 512))]
    if w > 512:
        out.append((512, w))
    return out


def build_nc():
    import ml_dtypes

    nc = bacc.Bacc("TRN2", target_bir_lowering=False, debug=False)

    cblob = nc.dram_tensor("cblob", (128, 952), BF16,
                           kind="ExternalInput").ap()
    xqT = nc.dram_tensor("xqT", (S, S), BF16, kind="ExternalInput").ap()
    xkT = nc.dram_tensor("xkT", (S, S), BF16, kind="ExternalInput").ap()
    xvT = nc.dram_tensor("xvT", (S, S), BF16, kind="ExternalInput").ap()
    wq = nc.dram_tensor("wq", (S, 512), BF16, kind="ExternalInput").ap()
    wk = nc.dram_tensor("wk", (S, 512), BF16, kind="ExternalInput").ap()
    wv = nc.dram_tensor("wv", (S, 512), BF16, kind="ExternalInput").ap()
    wo = nc.dram_tensor("wo", (512, S), BF16, kind="ExternalInput").ap()

    o_part = nc.dram_tensor("o_part", (S, S), BF16, kind="ExternalOutput").ap()
    ctx_out = nc.dram_tensor("ctx_out", (S, 512), BF16,
                             kind="ExternalOutput").ap()

    # greedy 3-way ACT/DVE/GPSIMD balance for PSUM->SBUF evictions,
    # preloaded with each engine's fixed work (ACT: exp; DVE: band adds,
    # P row sums, reciprocals, normalize-multiplies; GPSIMD: template
    # fills, band adds).
    # greedy ACT/DVE balance for PSUM->SBUF evictions (GPSIMD cannot touch
    # PSUM on hardware), preloaded with each engine's fixed work (ACT: exp;
    # DVE: reciprocals + normalize-multiplies).
    ebusy = {"act": 54500.0, "dve": 6400.0}

    def _pick(cols):
        costs = {
            "act": cols * 0.833 + 185.0 + 60.0,
            "dve": cols * 1.042 + 125.0 + 70.0,
        }
        eng = min(costs, key=lambda e: ebusy[e] + costs[e])
        ebusy[eng] += costs[eng]
        return eng

    def ecopy(out, in_, cols):
        if _pick(cols) == "act":
            nc.scalar.copy(out, in_)
        else:
            nc.vector.tensor_copy(out, in_)

    def ebias(out, in_, bias, cols):
        if _pick(cols) == "act":
            nc.scalar.activation(out, in_, AF.Identity, bias=bias)
        else:
            nc.vector.tensor_scalar_add(out, in_, bias)

    with tile.TileContext(nc) as tc:
        import contextlib

        with contextlib.ExitStack() as ctx:
            ep = ctx.enter_context
            cpool = ep(tc.tile_pool(name="consts", bufs=1))

            big = ep(tc.tile_pool(name="big", bufs=1))
            qT = big.tile([128, NPAIR, S], BF16, tag="qT", name="qT")[:]
            kT = big.tile([128, NPAIR, S], BF16, tag="kT", name="kT")[:]
            vN = big.tile([128, SB, 512], BF16, tag="vN", name="vN")[:]
            stg_h = []
            for _i in range(NHG):
                _t = big.tile([128, SB, WIN], BF16, tag=f"stg{_i}",
                              name=f"stg{_i}")
                stg_h.append(_t[:])
            srcb8_h = []
            for _i in range(NHG):
                _t = big.tile([128, SB, 20], BF16, tag=f"sb8{_i}",
                              name=f"sb8{_i}")
                srcb8_h.append(_t[:])
            sums8 = big.tile([128, SB, NHG], F32, tag="sums8",
                             name="sums8")[:]
            rjt = big.tile([128, SB, NHG], F32, tag="rjt", name="rjt")[:]
            cu = big.tile([64, NHG, 2, 512], BF16, tag="cu", name="cu")[:]

            # PSUM pools: 2*2 (sp/op) + 2*1 (gp) + 2*1 (av) = 8 banks
            spp = ep(tc.tile_pool(name="spp", bufs=2, space="PSUM"))
            gpp = ep(tc.tile_pool(name="gpp", bufs=2, space="PSUM"))
            avp = ep(tc.tile_pool(name="avp", bufs=2, space="PSUM"))

            # SBUF working pools
            ppool = ep(tc.tile_pool(name="ppool", bufs=4))
            ptp = ep(tc.tile_pool(name="ptp", bufs=8))
            cns = ep(tc.tile_pool(name="cns", bufs=2))
            ctp = ep(tc.tile_pool(name="ctp", bufs=2))
            osb = ep(tc.tile_pool(name="osb", bufs=2))
            xTp = ep(tc.tile_pool(name="xTp", bufs=4))
            wxp = ep(tc.tile_pool(name="wxp", bufs=3))

            def load_input(xdram, wdram, chunked_w=False):
                w_sb = wxp.tile([128, HC, 512], BF16, tag="wx",
                                name="w_sb")[:]
                xT = xTp.tile([128, HC, S], BF16, tag="xT", name="xT")[:]
                if chunked_w:
                    # interleave w/x chunk loads so the first projection
                    # matmuls start as soon as chunk 0 of each lands
                    for hc in range(HC):
                        nc.sync.dma_start(
                            w_sb[:, hc, :],
                            wdram[hc * 128:(hc + 1) * 128, :])
                        nc.sync.dma_start(xT[:, hc, :],
                                          xdram[hc * 128:(hc + 1) * 128, :])
                else:
                    nc.sync.dma_start(
                        w_sb, wdram.rearrange("(c p) n -> p c n", p=128))
                    for hc in range(HC):
                        nc.sync.dma_start(xT[:, hc, :],
                                          xdram[hc * 128:(hc + 1) * 128, :])
                return xT, w_sb

            def proj_pair(xT, w_sb, outT, bbase, pair, qc):
                pp = gpp.tile([128, 512], F32, tag="gp", name="pp")
                for hc in range(HC):
                    nc.tensor.matmul(
                        pp[:],
                        w_sb[:, hc, pair * 128:(pair + 1) * 128],
                        xT[:, hc, qc * 512:(qc + 1) * 512],
                        start=(hc == 0), stop=(hc == HC - 1))
                ebias(outT[:, pair, qc * 512:(qc + 1) * 512], pp[:],
                      bias_ap(bbase, pair), 512)

            # ---------------- Phase A: loads + projections + pre-pass ------
            cb = cpool.tile([128, 952], BF16, tag="cb")
            nc.sync.dma_start(cb[:], cblob)

            def bias_ap(base, pair):
                return cb[:, base + 2 * pair:base + 2 * pair + 2].bitcast(F32)

            xTq, w_q = load_input(xqT, wq, chunked_w=True)
            xTk, w_k = load_input(xkT, wk)

            # pair-0 projections only; pairs 1-3 are woven into the
            # attention groups as PE filler. One pair-1 chunk is emitted
            # here as PE filler while the K input streams in.
            proj_pair(xTq, w_q, qT, 296, 0, 0)
            proj_pair(xTq, w_q, qT, 296, 0, 1)
            proj_pair(xTq, w_q, qT, 296, 1, 0)
            proj_pair(xTk, w_k, kT, 304, 0, 0)
            proj_pair(xTk, w_k, kT, 304, 0, 1)

            # causal-mask templates for all (head, q-tile) staging tiles
            for h in range(NHG):
                for t in range(SB):
                    nc.gpsimd.tensor_copy(stg_h[h][:, t, :], cb[:, 0:WIN])

            # band pre-pass: srcb8[h][:, t, i] = 8*(rel[39-i]-rel[40]).Q[q]
            def prepass_mm(h):
                pairb, halfb = divmod(h, 2)
                qTh = qT[64 * halfb:64 * halfb + 64]
                for tg in (0, 4):
                    bp = gpp.tile([128, 512], F32, tag="gp", name="bp")
                    for tt in range(4):
                        t = tg + tt
                        nc.tensor.matmul(
                            bp[:, tt * 20:tt * 20 + 20],
                            qTh[:, pairb, t * 128:(t + 1) * 128],
                            cb[64 * halfb:64 * halfb + 64, WIN:WIN + 20],
                            start=True, stop=True)
                    ecopy(srcb8_h[h][:, tg:tg + 4, :], bp[:, 0:80], 80)

            def prepass_diag(h):
                stga = stg_h[h]
                diag = bass.AP(stga.tensor, stga.offset,
                               [[SB * WIN + 1, 128], [WIN, SB], [1, 20]])
                nc.sync.dma_start(diag, srcb8_h[h])

            def prepass(h):
                prepass_mm(h)
                prepass_diag(h)

            prepass(0)
            prepass(1)

            # V projection inputs; the projection matmuls themselves are
            # woven into the first attention group as PE filler.
            xTv, w_v = load_input(xvT, wv)

            def vproj(kb):
                pp = gpp.tile([128, 512], F32, tag="gp", name="pp")
                for hc in range(HC):
                    nc.tensor.matmul(
                        pp[:],
                        xTv[:, hc, kb * 128:(kb + 1) * 128],
                        w_v[:, hc, :],
                        start=(hc == 0), stop=False)
                nc.tensor.matmul(pp[:], cb[0:1, 824:952], cb[0:1, 312:824],
                                 start=False, stop=True)
                ecopy(vN[:, kb, :], pp[:], 512)

            # ---------------- Phase C worker (emitted interleaved) ---------
            # stores are emitted one tile late so they never head-of-line
            # block the SP DMA queue (which also carries the P transposes)
            pending_stores = []

            def flush_stores():
                while pending_stores:
                    pending_stores.pop(0)()

            def output_tile(t):
                nc.vector.reciprocal(rjt[:, t, :], sums8[:, t, :])
                cnall = gpp.tile([128, 512], BF16, tag="gp", name="cnall")
                tg, tl = divmod(t, 4)
                for h in range(NHG):
                    nc.tensor.transpose(
                        cnall[:, h * 64:(h + 1) * 64],
                        cu[:, h, tg, tl * 128:(tl + 1) * 128],
                        cb[0:64, 167:231])
                cn = cns.tile([128, 512], BF16, tag="cn")
                rj = rjt[:, t, :]
                rj_b = bass.AP(rj.tensor, rj.offset,
                               [[SB * NHG, 128], [1, NHG], [0, 64]])
                nc.vector.tensor_tensor(cn[:], cnall[:], rj_b, ALU.mult)
                flush_stores()
                pending_stores.append(
                    lambda cn=cn, t=t: nc.sync.dma_start(
                        ctx_out[t * 128:(t + 1) * 128, :], cn[:]))

                rt = gpp.tile([128, 512], BF16, tag="gp", name="rt")
                for pc in range(NPAIR):
                    nc.tensor.transpose(
                        rt[:, pc * 128:(pc + 1) * 128],
                        cn[:, pc * 128:(pc + 1) * 128],
                        cb[:, 167:295])
                ctxT = ctp.tile([128, NPAIR, 128], BF16, tag="ctxT")
                ecopy(ctxT[:], rt[:], 512)
                op = spp.tile([128, 1024], F32, tag="sp", name="op")
                for pc in range(NPAIR):
                    for oc in range(2):
                        nc.tensor.matmul(
                            op[:, oc * 512:(oc + 1) * 512],
                            ctxT[:, pc, :],
                            wo_sb[:, pc, oc * 512:(oc + 1) * 512],
                            start=(pc == 0), stop=(pc == NPAIR - 1))
                ou = osb.tile([128, 1024], BF16, tag="ou")
                ecopy(ou[:, 0:512], op[:, 0:512], 512)
                ecopy(ou[:, 512:1024], op[:, 512:1024], 512)
                pending_stores.append(
                    lambda ou=ou, t=t: nc.sync.dma_start(
                        o_part[t * 128:(t + 1) * 128, :], ou[:]))

            # ---------------- Phase B: attention --------------------------
            # head pairs run in lockstep over q-tiles so PE works on one
            # head's QK/AV while ACT runs the other head's exp. The first
            # output tiles are emitted into the last pair's tail so PE has
            # filler work while the exp->transpose->AV pipeline drains.
            DEFER = 6
            avs = {}
            p2s = {}
            pts = {}

            def qk_exp(h, t):
                W = 128 * (t + 1)
                pair, half = divmod(h, 2)
                qTh = qT[64 * half:64 * half + 64]
                kTh = kT[64 * half:64 * half + 64]
                sp = spp.tile([128, 1024], F32, tag="sp", name="sp")
                lhs = qTh[:, pair, t * 128:(t + 1) * 128]
                stga = stg_h[h][:, t, :]
                # QK scores with the causal template + rel-position band
                # accumulated in-PSUM via identity matmuls. Column segments
                # split at the band edges and PSUM bank boundary so every
                # column sees one accumulation group ending stop=True.
                if t == 0:
                    w0, soff, bandw = 0, 19, 128
                else:
                    w0 = t * 128 - 19
                    soff, bandw = -w0, WIN
                bounds = {0, w0, min(w0 + bandw, W), W}
                if W > 512:
                    bounds.add(512)
                bl = sorted(bounds)
                for a, b in zip(bl, bl[1:]):
                    if a >= b:
                        continue
                    inband = w0 <= a < w0 + bandw
                    nc.tensor.matmul(sp[:, a:b], lhs, kTh[:, pair, a:b],
                                     start=True, stop=not inband)
                    if inband:
                        nc.tensor.matmul(sp[:, a:b], cb[:, 167:295],
                                         stga[:, a + soff:b + soff],
                                         start=False, stop=True)
                # even/odd q-tiles of a head share one P tile so both are
                # transposed by a single XBAR DMA (halves SP-queue and
                # HWDGE pressure from the transpose path)
                if t % 2 == 0:
                    P2 = ppool.tile([128, 1920], BF16, tag="P", name="P")
                    p2s[h] = P2
                    off = 0
                else:
                    P2 = p2s[h]
                    off = 128 * t
                nc.scalar.activation(P2[:, off:off + W], sp[:, 0:W], AF.Exp,
                                     scale=1.0 / 64.0,
                                     accum_out=sums8[:, t, h:h + 1])
                if t % 2 == 1:
                    nblk = 2 * t + 1
                    pT = ptp.tile([128, 15, 128], BF16, tag="pT",
                                  name="pT")[:]
                    nc.sync.dma_start_transpose(pT[:, 0:nblk, :],
                                                P2[:, 0:off + W])
                    pts[h, t - 1] = (pT, 0)
                    pts[h, t] = (pT, t)

            def av_mm(h, t):
                tg, tl = divmod(t, 4)
                if tl == 0:
                    avs[h] = avp.tile([64, 512], F32, tag="av", name="av")
                av = avs[h]
                pT, boff = pts.pop((h, t))
                for kb in range(t + 1):
                    nc.tensor.matmul(
                        av[:, tl * 128:(tl + 1) * 128],
                        vN[:, kb, h * 64:(h + 1) * 64],
                        pT[:, boff + kb, :],
                        start=(kb == 0), stop=(kb == t))
                if tl == 3:
                    ecopy(cu[:, h, tg, :], av[:], 512)

            # PE filler work woven into each group's q-tile loop: the
            # remaining Q/K projection pairs, the V projection, the band
            # pre-passes for later heads, and the first output tiles.
            fillers = {
                0: [lambda: proj_pair(xTq, w_q, qT, 296, 1, 1),
                    lambda: proj_pair(xTk, w_k, kT, 304, 1, 0),
                    lambda: proj_pair(xTk, w_k, kT, 304, 1, 1),
                    lambda: prepass_mm(2)]
                   + [lambda kb=kb: vproj(kb) for kb in range(4)]
                   + [lambda: prepass_diag(2), lambda: prepass_mm(3)]
                   + [lambda kb=kb: vproj(kb) for kb in range(4, SB)]
                   + [lambda: prepass_diag(3)],
                1: [lambda qc=qc: proj_pair(xTq, w_q, qT, 296, 2, qc)
                    for qc in range(2)]
                   + [lambda qc=qc: proj_pair(xTk, w_k, kT, 304, 2, qc)
                      for qc in range(2)]
                   + [lambda: prepass_mm(4), lambda: prepass_diag(4),
                      lambda: prepass_mm(5), lambda: prepass_diag(5)],
                2: [lambda qc=qc: proj_pair(xTq, w_q, qT, 296, 3, qc)
                    for qc in range(2)]
                   + [lambda qc=qc: proj_pair(xTk, w_k, kT, 304, 3, qc)
                      for qc in range(2)]
                   + [lambda: prepass_mm(6), lambda: prepass_diag(6),
                      lambda: prepass_mm(7), lambda: prepass_diag(7)],
                3: [lambda tt=tt: output_tile(tt) for tt in range(4)],
            }
            # one continuous software pipeline over all (group, q-tile)
            # rounds: AVs trail their QK/exp by DEFER rounds across group
            # boundaries, so the PE stream never drains at a transition.
            seq = [(g, t) for g in range(NPAIR) for t in range(SB)]
            NR = len(seq)
            for i in range(NR + DEFER):
                if i < NR:
                    g, t = seq[i]
                    for h in (2 * g, 2 * g + 1):
                        qk_exp(h, t)
                if i >= DEFER:
                    g2, t2 = seq[i - DEFER]
                    for h in (2 * g2, 2 * g2 + 1):
                        av_mm(h, t2)
                gq = min(i // SB, NPAIR - 1)
                fl = fillers[gq]
                if gq < NPAIR - 1:
                    npop = -(-len(fl) // (SB - i % SB))
                    for _ in range(npop):
                        fl.pop(0)()
                elif i > 3 * SB + 3 + DEFER and fl:
                    # output tile T needs the tg0 AV evictions of ALL heads
                    fl.pop(0)()
                if i == 2 * SB:
                    # wo load deferred here to keep early DMA lanes clear;
                    # it reuses a free x slot
                    wo_sb = xTp.tile([128, NPAIR, S], BF16, tag="xT",
                                     name="wo_sb")[:]
                    nc.sync.dma_start(
                        wo_sb, wo.rearrange("(c p) n -> p c n", p=128))

            # ---------------- Phase C: remaining output tiles --------------
            for f in fillers[NPAIR - 1]:
                f()
            for t in range(4, SB):
                output_tile(t)
            flush_stores()

    nc.compile()
    return nc


_NC = None


def _get_nc():
    global _NC
    if _NC is None:
        _NC = build_nc()
    return _NC


def make_in_maps(query, key, value, Wq, bq, Wk, bk, Wv, bv, Wo, rel_emb):
    import ml_dtypes

    BF = ml_dtypes.bfloat16
    asb = lambda a: np.ascontiguousarray(np.asarray(a, np.float32)).astype(BF)
    asf = lambda a: np.ascontiguousarray(a, dtype=np.float32)
    rel = np.asarray(rel_emb, np.float32)
    # relTr[d + 64*dup, i] = 8*(rel[39-i, d] - rel[40, d])
    r1 = 8.0 * (rel[39::-1][:20].T - rel[40][:, None])  # (64, 20)
    relTr = asb(np.concatenate([r1, r1], axis=0))
    templ_np = np.zeros((128, WIN), dtype=np.float32)
    for p in range(128):
        templ_np[p, p + 20:] = MASKV

    def blob(bq_c, bk_c, bv_c):
        b = np.zeros((128, 952), BF)
        b[:, 0:WIN] = templ_np.astype(BF)
        b[:, WIN:WIN + 20] = relTr
        b[:, 167:295] = np.eye(128, dtype=np.float32).astype(BF)
        b[:, 296:304] = asf(bq_c.reshape(4, 128).T).view(BF)
        b[:, 304:312] = asf(bk_c.reshape(4, 128).T).view(BF)
        b[0, 312:824] = asb(bv_c)
        b[0, 824:952] = np.ones(128, BF)
        return b

    in_maps = []
    for c in range(8):
        n, hg = divmod(c, 2)
        cs = slice(512 * hg, 512 * (hg + 1))
        in_maps.append({
            "cblob": blob(np.asarray(bq)[cs], np.asarray(bk)[cs],
                          np.asarray(bv)[cs]),
            "xqT": asb(np.asarray(query[n]).T),
            "xkT": asb(np.asarray(key[n]).T),
            "xvT": asb(np.asarray(value[n]).T),
            "wq": asb(Wq[:, cs]),
            "wk": asb(Wk[:, cs]),
            "wv": asb(Wv[:, cs]),
            "wo": asb(Wo[cs, :]),
        })
    return in_maps


def run(inputs, trace=False, trace_kwargs=None):
    nc = _get_nc()
    in_maps = make_in_maps(
        np.asarray(inputs["query"]), np.asarray(inputs["key"]),
        np.asarray(inputs["value"]), np.asarray(inputs["Wq"]),
        np.asarray(inputs["bq"]), np.asarray(inputs["Wk"]),
        np.asarray(inputs["bk"]), np.asarray(inputs["Wv"]),
        np.asarray(inputs["bv"]), np.asarray(inputs["Wo"]),
        np.asarray(inputs["rel_emb"]))
    kw = {}
    if trace:
        kw["trace"] = True
        if trace_kwargs:
            kw.update(trace_kwargs)
    res = run_bass_kernel_spmd(nc, in_maps, core_ids=list(range(8)), **kw)
    bo = np.asarray(inputs["bo"], dtype=np.float32)
    out = np.zeros((4, S, S), np.float32)
    ctx = np.zeros((4, S, S), np.float32)
    for c in range(8):
        n, hg = divmod(c, 2)
        out[n] += np.asarray(res.results[c]["o_part"], np.float32)
        ctx[n][:, 512 * hg:512 * (hg + 1)] = np.asarray(
            res.results[c]["ctx_out"], np.float32)
    out += bo
    return (out, ctx), res


def kernel(**inputs):
    (out, ctx), _ = run(inputs)
    return (out, ctx)


# revision 81
# speedup vs baseline: 1.7821x; 1.0889x over previous
"""Trainium2 Bass kernel for MultiHeadAttention with relative-position bias.

Problem shapes: N=4, S=1024, H=1024, NH=16, D=64, P=20 (clamp window).
Returns (out, ctx) like the reference.

Sharding: 8 cores; core c handles batch n=c//2, head-group hg=c%2 (8 heads).
Each core computes its heads' QKV projections, attention, the ctx column
slice, and a partial out (row-sharded Wo contraction). Host sums the two
partials per batch and adds bo.

Design notes (vs the fp32r baseline):
  - bf16 data path end to end (fp32 PSUM accumulation); Q/K projections run
    in fp8e4m3 with DoubleRow perf mode (2 h-chunks per matmul at 2x rate) -
    safe because the attention energies are tiny, so Q/K noise is strongly
    damped by the softmax.
  - P^T for the AV contraction comes from one XBAR dma_start_transpose per
    (head, q-tile pair) instead of 320 PE transposes + 80 PSUM evictions.
  - The far-field rel-position term is dropped entirely (constant across
    keys for a fixed query, so softmax cancels it); the causal template and
    the 20-wide near-diagonal band are accumulated onto the QK scores
    in-PSUM by identity matmuls (start=False rides has_written), with one
    diagonal-AP DMA per head placing the band values.
  - Softmax row sums ride the exp activation's accum_out; normalization is
    one broadcast multiply per q-tile at the ctx transpose.
  - Attention runs as 4 head-pair groups with QK/exp emitted DEFER=6
    q-tiles ahead of the AV matmuls; projections for later pairs, the V
    projection, band pre-passes and the first output tiles are woven in as
    PE filler; output stores are emitted one tile late to avoid
    head-of-line blocking the SP DMA queue.
"""

import sys

if "/opt/trn_rl_repo" not in sys.path:
    sys.path.insert(0, "/opt/trn_rl_repo")

import numpy as np

import concourse.bass as bass
import concourse.mybir as mybir
import concourse.tile as tile
from concourse import bacc
from concourse.bass_utils import run_bass_kernel_spmd

F32 = mybir.dt.float32
BF16 = mybir.dt.bfloat16
FP8 = mybir.dt.float8e4
AF = mybir.ActivationFunctionType
ALU = mybir.AluOpType
AXIS = mybir.AxisListType

S = 1024
D = 64
NHG = 8      # heads per core
NPAIR = 4    # head pairs per core
HC = 8       # 128-row contraction chunks over H
SB = 8       # 128-row blocks over S
MASKV = -1.0e9
WIN = 147    # band window width (19 + 128)


def _chunks(w):
    out = [(0, min(w, 512))]
    if w > 512:
        out.append((512, w))
    return out


def build_nc():
    import ml_dtypes

    nc = bacc.Bacc("TRN2", target_bir_lowering=False, debug=False)

    cblob = nc.dram_tensor("cblob", (128, 952), BF16,
                           kind="ExternalInput").ap()
    xqT = nc.dram_tensor("xqT", (S, S), FP8, kind="ExternalInput").ap()
    xkT = nc.dram_tensor("xkT", (S, S), FP8, kind="ExternalInput").ap()
    xvT = nc.dram_tensor("xvT", (S, S), BF16, kind="ExternalInput").ap()
    wq = nc.dram_tensor("wq", (S, 512), FP8, kind="ExternalInput").ap()
    wk = nc.dram_tensor("wk", (S, 512), FP8, kind="ExternalInput").ap()
    wv = nc.dram_tensor("wv", (S, 512), BF16, kind="ExternalInput").ap()
    wo = nc.dram_tensor("wo", (512, S), BF16, kind="ExternalInput").ap()

    o_part = nc.dram_tensor("o_part", (S, S), BF16, kind="ExternalOutput").ap()
    ctx_out = nc.dram_tensor("ctx_out", (S, 512), BF16,
                             kind="ExternalOutput").ap()

    # greedy 3-way ACT/DVE/GPSIMD balance for PSUM->SBUF evictions,
    # preloaded with each engine's fixed work (ACT: exp; DVE: band adds,
    # P row sums, reciprocals, normalize-multiplies; GPSIMD: template
    # fills, band adds).
    # greedy ACT/DVE balance for PSUM->SBUF evictions (GPSIMD cannot touch
    # PSUM on hardware), preloaded with each engine's fixed work (ACT: exp;
    # DVE: reciprocals + normalize-multiplies).
    ebusy = {"act": 70000.0, "dve": 6400.0}

    def _pick(cols):
        costs = {
            "act": cols * 0.833 + 185.0 + 60.0,
            "dve": cols * 1.042 + 125.0 + 70.0,
        }
        eng = min(costs, key=lambda e: ebusy[e] + costs[e])
        ebusy[eng] += costs[eng]
        return eng

    def ecopy(out, in_, cols):
        if _pick(cols) == "act":
            nc.scalar.copy(out, in_)
        else:
            nc.vector.tensor_copy(out, in_)

    def ebias(out, in_, bias, cols):
        if _pick(cols) == "act":
            nc.scalar.activation(out, in_, AF.Identity, bias=bias)
        else:
            nc.vector.tensor_scalar_add(out, in_, bias)

    with tile.TileContext(nc) as tc:
        import contextlib

        with contextlib.ExitStack() as ctx:
            ep = ctx.enter_context
            cpool = ep(tc.tile_pool(name="consts", bufs=1))

            big = ep(tc.tile_pool(name="big", bufs=1))
            qT = big.tile([128, NPAIR, S], BF16, tag="qT", name="qT")[:]
            kT = big.tile([128, NPAIR, S], BF16, tag="kT", name="kT")[:]
            vN = big.tile([128, SB, 512], BF16, tag="vN", name="vN")[:]
            stg_h = []
            for _i in range(NHG):
                _t = big.tile([128, SB, WIN], BF16, tag=f"stg{_i}",
                              name=f"stg{_i}")
                stg_h.append(_t[:])
            srcb8_h = []
            for _i in range(NHG):
                _t = big.tile([128, SB, 20], BF16, tag=f"sb8{_i}",
                              name=f"sb8{_i}")
                srcb8_h.append(_t[:])
            sums8 = big.tile([128, SB, NHG], F32, tag="sums8",
                             name="sums8")[:]
            rjt = big.tile([128, SB, NHG], F32, tag="rjt", name="rjt")[:]
            cu = big.tile([128, NPAIR, 2, 512], BF16, tag="cu", name="cu")[:]

            # PSUM pools: 2*2 (sp/op) + 2*1 (gp) + 2*1 (av) = 8 banks
            spp = ep(tc.tile_pool(name="spp", bufs=2, space="PSUM"))
            gpp = ep(tc.tile_pool(name="gpp", bufs=2, space="PSUM"))
            avp = ep(tc.tile_pool(name="avp", bufs=2, space="PSUM"))

            # SBUF working pools
            ppool = ep(tc.tile_pool(name="ppool", bufs=4))
            ptp = ep(tc.tile_pool(name="ptp", bufs=8))
            cns = ep(tc.tile_pool(name="cns", bufs=2))
            osb = ep(tc.tile_pool(name="osb", bufs=2))
            xTp = ep(tc.tile_pool(name="xTp", bufs=4))
            wxp = ep(tc.tile_pool(name="wxp", bufs=3))

            def load_input(xdram, wdram, chunked_w=False, dtype=BF16):
                w_sb = wxp.tile([128, HC, 512], dtype, tag="wx",
                                name="w_sb")[:]
                xT = xTp.tile([128, HC, S], dtype, tag="xT", name="xT")[:]
                if chunked_w:
                    # interleave w/x chunk loads so the first projection
                    # matmuls start as soon as chunk 0 of each lands
                    for hc in range(HC):
                        nc.sync.dma_start(
                            w_sb[:, hc, :],
                            wdram[hc * 128:(hc + 1) * 128, :])
                        nc.sync.dma_start(xT[:, hc, :],
                                          xdram[hc * 128:(hc + 1) * 128, :])
                else:
                    nc.sync.dma_start(
                        w_sb, wdram.rearrange("(c p) n -> p c n", p=128))
                    for hc in range(HC):
                        nc.sync.dma_start(xT[:, hc, :],
                                          xdram[hc * 128:(hc + 1) * 128, :])
                return xT, w_sb

            def proj_pair(xT, w_sb, outT, bbase, pair, qc):
                # fp8 DoubleRow: two h-chunks per matmul at 2x rate
                pp = gpp.tile([128, 512], F32, tag="gp", name="pp")
                for j in range(HC // 2):
                    nc.tensor.matmul(
                        pp[:],
                        w_sb[:, 2 * j:2 * j + 2,
                             pair * 128:(pair + 1) * 128],
                        xT[:, 2 * j:2 * j + 2,
                           qc * 512:(qc + 1) * 512],
                        start=(j == 0), stop=(j == HC // 2 - 1),
                        perf_mode=mybir.MatmulPerfMode.DoubleRow)
                ebias(outT[:, pair, qc * 512:(qc + 1) * 512], pp[:],
                      bias_ap(bbase, pair), 512)

            # ---------------- Phase A: loads + projections + pre-pass ------
            cb = cpool.tile([128, 952], BF16, tag="cb")
            nc.sync.dma_start(cb[:], cblob)

            def bias_ap(base, pair):
                return cb[:, base + 2 * pair:base + 2 * pair + 2].bitcast(F32)

            xTq, w_q = load_input(xqT, wq, chunked_w=True, dtype=FP8)
            xTk, w_k = load_input(xkT, wk, dtype=FP8)

            # pair-0 projections only; pairs 1-3 are woven into the
            # attention groups as PE filler. One pair-1 chunk is emitted
            # here as PE filler while the K input streams in.
            proj_pair(xTq, w_q, qT, 296, 0, 0)
            proj_pair(xTq, w_q, qT, 296, 0, 1)
            proj_pair(xTq, w_q, qT, 296, 1, 0)
            proj_pair(xTk, w_k, kT, 304, 0, 0)
            proj_pair(xTk, w_k, kT, 304, 0, 1)

            # causal-mask templates for all (head, q-tile) staging tiles
            for h in range(NHG):
                for t in range(SB):
                    nc.gpsimd.tensor_copy(stg_h[h][:, t, :], cb[:, 0:WIN])

            # band pre-pass: srcb8[h][:, t, i] = 8*(rel[39-i]-rel[40]).Q[q]
            def prepass_mm(h):
                pairb, halfb = divmod(h, 2)
                qTh = qT[64 * halfb:64 * halfb + 64]
                for tg in (0, 4):
                    bp = gpp.tile([128, 512], F32, tag="gp", name="bp")
                    for tt in range(4):
                        t = tg + tt
                        nc.tensor.matmul(
                            bp[:, tt * 20:tt * 20 + 20],
                            qTh[:, pairb, t * 128:(t + 1) * 128],
                            cb[64 * halfb:64 * halfb + 64, WIN:WIN + 20],
                            start=True, stop=True)
                    ecopy(srcb8_h[h][:, tg:tg + 4, :], bp[:, 0:80], 80)

            def prepass_diag(h):
                stga = stg_h[h]
                diag = bass.AP(stga.tensor, stga.offset,
                               [[SB * WIN + 1, 128], [WIN, SB], [1, 20]])
                nc.sync.dma_start(diag, srcb8_h[h])

            def prepass(h):
                prepass_mm(h)
                prepass_diag(h)

            prepass(0)
            prepass(1)

            # V projection inputs; the projection matmuls themselves are
            # woven into the first attention group as PE filler.
            xTv, w_v = load_input(xvT, wv)

            def vproj(kb):
                pp = gpp.tile([128, 512], F32, tag="gp", name="pp")
                for hc in range(HC):
                    nc.tensor.matmul(
                        pp[:],
                        xTv[:, hc, kb * 128:(kb + 1) * 128],
                        w_v[:, hc, :],
                        start=(hc == 0), stop=False)
                nc.tensor.matmul(pp[:], cb[0:1, 824:952], cb[0:1, 312:824],
                                 start=False, stop=True)
                ecopy(vN[:, kb, :], pp[:], 512)

            # ---------------- Phase C worker (emitted interleaved) ---------
            # stores are emitted one tile late so they never head-of-line
            # block the SP DMA queue (which also carries the P transposes)
            pending_stores = []

            def flush_stores():
                while pending_stores:
                    pending_stores.pop(0)()

            def output_tile(t):
                # cu already holds NORMALIZED ctx^T (P was scaled by 1/sums
                # before AV), head pairs stacked on 128 partitions: the out
                # projection reads it directly, and ctx_out needs only the
                # [q,d] transposes.
                tg, tl = divmod(t, 4)
                op = spp.tile([128, 1024], F32, tag="sp", name="op")
                for pc in range(NPAIR):
                    for oc in range(2):
                        nc.tensor.matmul(
                            op[:, oc * 512:(oc + 1) * 512],
                            cu[:, pc, tg, tl * 128:(tl + 1) * 128],
                            wo_sb[:, pc, oc * 512:(oc + 1) * 512],
                            start=(pc == 0), stop=(pc == NPAIR - 1))
                cnall = gpp.tile([128, 512], BF16, tag="gp", name="cnall")
                for pc in range(NPAIR):
                    nc.tensor.transpose(
                        cnall[:, pc * 128:(pc + 1) * 128],
                        cu[:, pc, tg, tl * 128:(tl + 1) * 128],
                        cb[:, 167:295])
                cn = cns.tile([128, 512], BF16, tag="cn")
                nc.scalar.copy(cn[:], cnall[:])
                flush_stores()
                pending_stores.append(
                    lambda cn=cn, t=t: nc.sync.dma_start(
                        ctx_out[t * 128:(t + 1) * 128, :], cn[:]))
                ou = osb.tile([128, 1024], BF16, tag="ou")
                nc.scalar.copy(ou[:, 0:512], op[:, 0:512])
                nc.vector.tensor_copy(ou[:, 512:1024], op[:, 512:1024])
                pending_stores.append(
                    lambda ou=ou, t=t: nc.sync.dma_start(
                        o_part[t * 128:(t + 1) * 128, :], ou[:]))

            # ---------------- Phase B: attention --------------------------
            # head pairs run in lockstep over q-tiles so PE works on one
            # head's QK/AV while ACT runs the other head's exp. The first
            # output tiles are emitted into the last pair's tail so PE has
            # filler work while the exp->transpose->AV pipeline drains.
            DEFER = 6
            avs = {}
            p2s = {}
            pts = {}

            def qk_exp(h, t):
                W = 128 * (t + 1)
                pair, half = divmod(h, 2)
                qTh = qT[64 * half:64 * half + 64]
                kTh = kT[64 * half:64 * half + 64]
                sp = spp.tile([128, 1024], F32, tag="sp", name="sp")
                lhs = qTh[:, pair, t * 128:(t + 1) * 128]
                stga = stg_h[h][:, t, :]
                # QK scores with the causal template + rel-position band
                # accumulated in-PSUM via identity matmuls. Column segments
                # split at the band edges and PSUM bank boundary so every
                # column sees one accumulation group ending stop=True.
                if t == 0:
                    w0, soff, bandw = 0, 19, 128
                else:
                    w0 = t * 128 - 19
                    soff, bandw = -w0, WIN
                bounds = {0, w0, min(w0 + bandw, W), W}
                if W > 512:
                    bounds.add(512)
                bl = sorted(bounds)
                for a, b in zip(bl, bl[1:]):
                    if a >= b:
                        continue
                    inband = w0 <= a < w0 + bandw
                    nc.tensor.matmul(sp[:, a:b], lhs, kTh[:, pair, a:b],
                                     start=True, stop=not inband)
                    if inband:
                        nc.tensor.matmul(sp[:, a:b], cb[:, 167:295],
                                         stga[:, a + soff:b + soff],
                                         start=False, stop=True)
                # even/odd q-tiles of a head share one P tile so both are
                # transposed by a single XBAR DMA (halves SP-queue and
                # HWDGE pressure from the transpose path)
                if t % 2 == 0:
                    P2 = ppool.tile([128, 1920], BF16, tag="P", name="P")
                    p2s[h] = P2
                    off = 0
                else:
                    P2 = p2s[h]
                    off = 128 * t
                nc.scalar.activation(P2[:, off:off + W], sp[:, 0:W], AF.Exp,
                                     scale=1.0 / 64.0,
                                     accum_out=sums8[:, t, h:h + 1])
                # normalize P in place so AV produces normalized ctx^T
                nc.vector.reciprocal(rjt[:, t, h:h + 1], sums8[:, t, h:h + 1])
                nc.vector.tensor_scalar_mul(P2[:, off:off + W],
                                            P2[:, off:off + W],
                                            rjt[:, t, h:h + 1])
                if t % 2 == 1:
                    nblk = 2 * t + 1
                    pT = ptp.tile([128, 15, 128], BF16, tag="pT",
                                  name="pT")[:]
                    nc.sync.dma_start_transpose(pT[:, 0:nblk, :],
                                                P2[:, 0:off + W])
                    pts[h, t - 1] = (pT, 0)
                    pts[h, t] = (pT, t)

            def av_mm(h, t):
                # head pairs share one AV PSUM bank: the odd head's output
                # lands at partitions 64-127 via the tile-position path, so
                # the eviction yields out-projection-ready [128, q] chunks
                g, odd = divmod(h, 2)
                tg, tl = divmod(t, 4)
                if tl == 0 and not odd:
                    avs[g] = avp.tile([128, 512], F32, tag="av", name="av")
                av = avs[g]
                pT, boff = pts.pop((h, t))
                rows = av[:][64:128, tl * 128:(tl + 1) * 128] if odd \
                    else av[:][0:64, tl * 128:(tl + 1) * 128]
                for kb in range(t + 1):
                    nc.tensor.matmul(
                        rows,
                        vN[:, kb, h * 64:(h + 1) * 64],
                        pT[:, boff + kb, :],
                        start=(kb == 0), stop=(kb == t))
                if tl == 3 and odd:
                    ecopy(cu[:, g, tg, :], av[:], 512)

            # PE filler work woven into each group's q-tile loop: the
            # remaining Q/K projection pairs, the V projection, the band
            # pre-passes for later heads, and the first output tiles.
            fillers = {
                0: [lambda: proj_pair(xTq, w_q, qT, 296, 1, 1),
                    lambda: proj_pair(xTk, w_k, kT, 304, 1, 0),
                    lambda: proj_pair(xTk, w_k, kT, 304, 1, 1),
                    lambda: prepass_mm(2)]
                   + [lambda kb=kb: vproj(kb) for kb in range(4)]
                   + [lambda: prepass_diag(2), lambda: prepass_mm(3)]
                   + [lambda kb=kb: vproj(kb) for kb in range(4, SB)]
                   + [lambda: prepass_diag(3)],
                1: [lambda qc=qc: proj_pair(xTq, w_q, qT, 296, 2, qc)
                    for qc in range(2)]
                   + [lambda qc=qc: proj_pair(xTk, w_k, kT, 304, 2, qc)
                      for qc in range(2)]
                   + [lambda: prepass_mm(4), lambda: prepass_diag(4),
                      lambda: prepass_mm(5), lambda: prepass_diag(5)],
                2: [lambda qc=qc: proj_pair(xTq, w_q, qT, 296, 3, qc)
                    for qc in range(2)]
                   + [lambda qc=qc: proj_pair(xTk, w_k, kT, 304, 3, qc)
                      for qc in range(2)]
                   + [lambda: prepass_mm(6), lambda: prepass_diag(6),
                      lambda: prepass_mm(7), lambda: prepass_diag(7)],
                3: [lambda tt=tt: output_tile(tt) for tt in range(4)],
            }
            for g in range(NPAIR):
                fl = list(fillers[g])
                nslots = SB + DEFER
                for t in range(SB + DEFER):
                    for h in (2 * g, 2 * g + 1):
                        if t < SB:
                            qk_exp(h, t)
                    for h in (2 * g, 2 * g + 1):
                        if t >= DEFER:
                            av_mm(h, t - DEFER)
                    if g == NPAIR - 1:
                        # output tile T needs the tg0 AV evictions of ALL
                        # heads, emitted at loop t = 3 + DEFER; only pop
                        # output fillers after that point
                        if t > 3 + DEFER and fl:
                            fl.pop(0)()
                        continue
                    npop = -(-len(fl) // (nslots - t))
                    for _ in range(npop):
                        fl.pop(0)()
                if g == 1:
                    # wo load deferred here to keep early DMA lanes clear;
                    # it reuses a free x slot
                    wo_sb = xTp.tile([128, NPAIR, S], BF16, tag="xT",
                                     name="wo_sb")[:]
                    nc.sync.dma_start(
                        wo_sb, wo.rearrange("(c p) n -> p c n", p=128))

            # ---------------- Phase C: remaining output tiles --------------
            for f in fl:
                f()
            for t in range(4, SB):
                output_tile(t)
            flush_stores()

    nc.compile()
    return nc


_NC = None


def _get_nc():
    global _NC
    if _NC is None:
        _NC = build_nc()
    return _NC


def make_in_maps(query, key, value, Wq, bq, Wk, bk, Wv, bv, Wo, rel_emb):
    import ml_dtypes

    BF = ml_dtypes.bfloat16
    F8 = ml_dtypes.float8_e4m3
    asb = lambda a: np.ascontiguousarray(np.asarray(a, np.float32)).astype(BF)
    as8 = lambda a: np.ascontiguousarray(np.asarray(a, np.float32)).astype(F8)
    asf = lambda a: np.ascontiguousarray(a, dtype=np.float32)
    rel = np.asarray(rel_emb, np.float32)
    # relTr[d + 64*dup, i] = 8*(rel[39-i, d] - rel[40, d])
    r1 = 8.0 * (rel[39::-1][:20].T - rel[40][:, None])  # (64, 20)
    relTr = asb(np.concatenate([r1, r1], axis=0))
    templ_np = np.zeros((128, WIN), dtype=np.float32)
    for p in range(128):
        templ_np[p, p + 20:] = MASKV

    def blob(bq_c, bk_c, bv_c):
        b = np.zeros((128, 952), BF)
        b[:, 0:WIN] = templ_np.astype(BF)
        b[:, WIN:WIN + 20] = relTr
        b[:, 167:295] = np.eye(128, dtype=np.float32).astype(BF)
        b[:, 296:304] = asf(bq_c.reshape(4, 128).T).view(BF)
        b[:, 304:312] = asf(bk_c.reshape(4, 128).T).view(BF)
        b[0, 312:824] = asb(bv_c)
        b[0, 824:952] = np.ones(128, BF)
        return b

    in_maps = []
    for c in range(8):
        n, hg = divmod(c, 2)
        cs = slice(512 * hg, 512 * (hg + 1))
        in_maps.append({
            "cblob": blob(np.asarray(bq)[cs], np.asarray(bk)[cs],
                          np.asarray(bv)[cs]),
            "xqT": as8(np.asarray(query[n]).T),
            "xkT": as8(np.asarray(key[n]).T),
            "xvT": asb(np.asarray(value[n]).T),
            "wq": as8(Wq[:, cs]),
            "wk": as8(Wk[:, cs]),
            "wv": asb(Wv[:, cs]),
            "wo": asb(Wo[cs, :]),
        })
    return in_maps


def run(inputs, trace=False, trace_kwargs=None):
    nc = _get_nc()
    in_maps = make_in_maps(
        np.asarray(inputs["query"]), np.asarray(inputs["key"]),
        np.asarray(inputs["value"]), np.asarray(inputs["Wq"]),
        np.asarray(inputs["bq"]), np.asarray(inputs["Wk"]),
        np.asarray(inputs["bk"]), np.asarray(inputs["Wv"]),
        np.asarray(inputs["bv"]), np.asarray(inputs["Wo"]),
        np.asarray(inputs["rel_emb"]))
    kw = {}
    if trace:
        kw["trace"] = True
        if trace_kwargs:
            kw.update(trace_kwargs)
    res = run_bass_kernel_spmd(nc, in_maps, core_ids=list(range(8)), **kw)
    bo = np.asarray(inputs["bo"], dtype=np.float32)
    out = np.zeros((4, S, S), np.float32)
    ctx = np.zeros((4, S, S), np.float32)
    for c in range(8):
        n, hg = divmod(c, 2)
        out[n] += np.asarray(res.results[c]["o_part"], np.float32)
        ctx[n][:, 512 * hg:512 * (hg + 1)] = np.asarray(
            res.results[c]["ctx_out"], np.float32)
    out += bo
    return (out, ctx), res


def kernel(**inputs):
    (out, ctx), _ = run(inputs)
    return (out, ctx)
